# revision 1
# baseline (speedup 1.0000x reference)
"""Trainium2 Bass kernel for nn_CustomModel_12953621365157 (gnn_message_passing).

Strategy
--------
Data-parallel over the batch axis: 8 cores, 512 batch columns each.

Per layer the reference does gather(edge_src) -> x0.5-weight-with-|g|==1-quirk
-> segment_sum(edge_dst) -> per-node activation.  Because the quirk depends
only on the gathered *value*, it folds into the source node:

    v_adj[n] = v[n] + c*(v[n]==1) - c*(v[n]==-1),   c = (1-w)/w
    summed   = w * (A_l @ v_adj)                    A_l[p,n] = #edges n->p

so each layer is a dense [P x N_l] x [N_l x B_s] matmul on the PE (float32r:
full-rate at N=512, ~1e-4 relative accuracy).  A_l is built host-side from the
edge lists.  Nodes of each layer are pre-sorted by activation id so the
per-node activation dispatch becomes a few partition-range ACT instructions
(cos = sin(x+pi/2), sigmoid = 0.5*tanh(x/2)+0.5, Gaussian = exp(-square(x)),
step on the DVE).  The sign trick: V tiles store -v_adj so the quirk chain is
2 fused scalar_tensor_tensor ops; all ACT scales are negated to compensate.
"""

import numpy as np

N_IN = 512
P = 512
L = 4
E = 32768
B = 4096
N_CORES = 8
BS = B // N_CORES  # 512 batch columns per core

# activation ids (order matches reference activations_dict)
LINEAR, STEP, SIN, COS, GAUSS, TANH, SIGMOID, ABS, INVERT, RELU = range(10)
# node sort order per layer: exp-set funcs first (gauss), then silu-set
# (sin/cos/tanh/sigmoid), then DVE/simple funcs.
FUNC_ORDER = [GAUSS, TANH, SIGMOID, STEP, ABS, RELU, LINEAR, INVERT, SIN, COS]
ORDER_RANK = {f: i for i, f in enumerate(FUNC_ORDER)}

TILE_BASE = [0, 4, 12, 24]  # first A-tile index of each layer; 40 tiles total
N_A_TILES = 40


def _preprocess(x, w, edge_src, edge_dst, act_ids):
    """Host-side: node sort per layer, dense A build, input quirk fold."""
    c = (1.0 - w) / w

    perms = []      # perms[l][p_sorted] = orig node j
    inv_perms = []  # inv_perms[l][orig j] = p_sorted
    segs = []       # segs[l][m] = list of (func_id, lo, hi) within chunk m
    for l in range(L):
        ids = np.asarray(act_ids[l])
        key = np.array([ORDER_RANK[int(i)] for i in ids])
        perm = np.argsort(key, kind="stable")
        inv = np.empty(P, np.int64)
        inv[perm] = np.arange(P)
        perms.append(perm)
        inv_perms.append(inv)
        ids_sorted = ids[perm]
        layer_segs = []
        for m in range(4):
            chunk = ids_sorted[m * 128:(m + 1) * 128]
            runs = []
            lo = 0
            for i in range(1, 129):
                if i == 128 or chunk[i] != chunk[lo]:
                    runs.append((int(chunk[lo]), lo, i))
                    lo = i
            layer_segs.append(runs)
        segs.append(layer_segs)

    inv_stack = np.stack(inv_perms)  # [L, P]

    a_pack = np.zeros((N_A_TILES * 128, P), np.float32)
    for l in range(L):
        src = np.asarray(edge_src[l]).astype(np.int64)
        dst = np.asarray(edge_dst[l]).astype(np.int64)
        g = src.copy()
        m = g >= N_IN
        lp = (g[m] - N_IN) // P
        j = (g[m] - N_IN) % P
        g[m] = N_IN + lp * P + inv_stack[lp, j]
        d = inv_perms[l][dst]
        np.add.at(a_pack, (TILE_BASE[l] * 128 + g, d), 1.0)

    xa = x.astype(np.float32)
    if c != 0.0:
        xa = xa + c * (xa == 1.0) - c * (xa == -1.0)
    xin = -xa  # V tiles hold -v_adj
    return a_pack, xin.astype(np.float32), perms, segs


DEBUG_DUMP = False
ACT_CHAIN = True

_QUIRK_OP = None


def _get_quirk_fold_op():
    """Custom single-uop DVE op: out = (in==-1) - ((in==1) + in) = -v_adj.

    Replaces the two scalar_tensor_tensor ops of the quirk chain.  NOTE:
    custom DVE ops only work on APs with partition base 0 (silently no-op
    otherwise), so this is only used full-tile.
    """
    global _QUIRK_OP
    if _QUIRK_OP is not None:
        return _QUIRK_OP
    import concourse.dve_ops as dve_ops
    from concourse.dve_spec import (Spec, Src0, C0, C1, Bin, AluOp, lower,
                                    _has_src1)
    from concourse.dve_uop import DveOpSpec

    def eq(a, b):
        return Bin(AluOp.IS_EQ, a, b)

    spec = Spec(
        body=eq(Src0, C1) - (eq(Src0, C0) + Src0),
        reference=lambda in0, s0, s1, imm2: (
            (in0 == s1).astype(np.float32)
            - ((in0 == s0).astype(np.float32) + in0)),
    )
    name = "QUIRK_FOLD_ANT"
    if name not in dve_ops._SUB_OPCODE_FOR_NAME:
        row = max(dve_ops._SUB_OPCODE_FOR_NAME.values()) + 1
        assert row < 0x20
        dve_ops._SUB_OPCODE_FOR_NAME[name] = row
    opcode = dve_ops._SUB_OPCODE_FOR_NAME[name]
    shas = {}
    for ver in ("v3", "v4"):
        u = lower(spec, ver=ver)
        shas[ver] = DveOpSpec(name=name, opcode=opcode, uops=u,
                              rd1_en=_has_src1(spec)).sha(ver)
    op = dve_ops.DveOp(name, spec, subdim=False, uops_sha=shas)
    if all(o.name != name for o in dve_ops.OPS):
        dve_ops.OPS.append(op)
    dve_ops.CUSTOM_DVE_SPECS[name] = spec
    _QUIRK_OP = op
    return op


def _build_program(segs, w):
    import concourse.bass as bass
    import concourse.bacc as bacc
    import concourse.mybir as mybir
    import concourse.tile as tile
    from concourse.tile_rust import add_dep_helper

    quirk_op = _get_quirk_fold_op()

    dt = mybir.dt
    Act = mybir.ActivationFunctionType
    Alu = mybir.AluOpType
    W = float(w)

    # Cody-Waite split of 2*pi: c1/c2 short so k*c1, k*c2 are exact for
    # |k| < 2^12; c3 carries the remainder.
    def _trunc(x, bits):
        u = np.float32(x).view(np.uint32)
        mask = np.uint32(0xFFFFFFFF) << np.uint32(23 - bits)
        return float((u & mask).view(np.float32))

    TWO_PI = 2.0 * np.pi
    CW1 = _trunc(TWO_PI, 7)                    # 6.28125, exact
    CW2 = _trunc(TWO_PI - CW1, 12)
    CW3 = float(np.float32(TWO_PI - CW1 - CW2))
    INV_2PI = float(np.float32(1.0 / TWO_PI))
    PI_F = float(np.float32(np.pi))
    HALF_PI = float(np.float32(np.pi / 2))
    TWO_PI_F = float(np.float32(TWO_PI))
    c = (1.0 - W) / W
    fast_chain = (W == 0.5)

    nc = bacc.Bacc("TRN2", target_bir_lowering=False, debug=False,
                   num_devices=N_CORES)
    xin = nc.dram_tensor("xin", [N_IN, BS], dt.float32r,
                         kind="ExternalInput").ap()
    a_d = nc.dram_tensor("amat", [N_A_TILES * 128, P], dt.float32r,
                         kind="ExternalInput").ap()
    out_d = nc.dram_tensor("out", [P, BS], dt.float32,
                           kind="ExternalOutput").ap()
    dbg = {}
    if DEBUG_DUMP:
        for nm in ["k", "z", "r", "wrap"]:
            dbg[nm] = nc.dram_tensor(f"dbg_{nm}", [128, BS], dt.float32,
                                     kind="ExternalOutput").ap()
        for l in range(L - 1):
            dbg[("vraw", l)] = nc.dram_tensor(
                f"dbg_vraw{l}", [P, BS], dt.float32, kind="ExternalOutput").ap()
            dbg[("vadj", l)] = nc.dram_tensor(
                f"dbg_vadj{l}", [P, BS], dt.float32, kind="ExternalOutput").ap()

    with tile.TileContext(nc) as tc:
        with tc.tile_pool(name="Ap", bufs=1) as apool, \
             tc.tile_pool(name="Vp", bufs=1) as vpool, \
             tc.tile_pool(name="raw", bufs=5) as rpool, \
             tc.tile_pool(name="ps", bufs=8, space="PSUM") as ppool:

            # input node values (already quirk-folded & negated on host)
            V = []
            for t in range(4):
                vt = vpool.tile([128, BS], dt.float32r, name=f"v{t}")
                nc.sync.dma_start(vt[:], xin[t * 128:(t + 1) * 128, :])
                V.append(vt)

            A = {}
            for l in range(L):
                for k in range(4 + 4 * l):
                    at = apool.tile([128, P], dt.float32r, name=f"a{l}_{k}")
                    r0 = (TILE_BASE[l] + k) * 128
                    nc.sync.dma_start(at[:], a_d[r0:r0 + 128, :])
                    A[(l, k)] = at

            for l in range(L):
                nk = 4 + 4 * l
                psums = []
                for m in range(4):
                    ps = ppool.tile([128, BS], dt.float32, name="ps")
                    for k in range(nk):
                        ak = A[(l, k)]
                        nc.tensor.matmul(
                            ps[:], ak[:, m * 128:(m + 1) * 128],
                            V[k][:], start=(k == 0), stop=(k == nk - 1))
                    psums.append(ps)

                # Engine instructions must start at partition 0/32/64/96
                # and not cross their aligned block end.  Each segment is
                # extended down to a 32-aligned start and split into valid
                # "buddy" pieces; segments are emitted in DESCENDING partition
                # order so the true owner of every overlap region writes last.
                # Gaussian (the only exp-table-set user) is emitted after all
                # silu-set segments of the layer: 2 ACT table loads per layer.
                def _pieces(lo, hi):
                    p = (lo // 32) * 32
                    out = []
                    while p < hi:
                        end = min(hi, 64) if p == 32 else hi
                        out.append((p, end))
                        p = end
                    return out

                vraws, tmps = [], []
                for m in range(4):
                    vraw = rpool.tile([128, BS], dt.float32, name="vraw")
                    tmp = rpool.tile([128, BS], dt.float32, name="tmp")
                    vraws.append(vraw)
                    tmps.append(tmp)
                # sin/cos need |arg| <= pi (the Sin spline only covers
                # |x| < 4): per chunk containing sin/cos nodes, compute the
                # Cody-Waite-reduced argument r = z - 2*pi*round-ish(z/2pi)
                # on the full tile (DVE cost is partition-count independent),
                # then each sin/cos segment wraps (+pi/2 for cos) into
                # [-pi, pi] and applies Sin with no bias.
                rtiles = {}
                for m in range(4):
                    if not any(f in (SIN, COS) for f, _, _ in segs[l][m]):
                        continue
                    ps = psums[m]
                    sq = rpool.tile([128, BS], dt.float32, name="sq", bufs=2)
                    si = rpool.tile([128, BS], dt.int32, name="si", bufs=2)
                    sz = rpool.tile([128, BS], dt.float32, name="sz", bufs=2)
                    sr = rpool.tile([128, BS], dt.float32, name="sr", bufs=2)
                    nc.vector.tensor_scalar(si[:], ps[:], -W * INV_2PI,
                                            None, Alu.mult)  # i32 out: k
                    nc.vector.tensor_copy(sq[:], si[:])   # i32 -> f32 (= k)
                    nc.vector.tensor_scalar(sz[:], ps[:], -W, None, Alu.mult)
                    nc.vector.cody_waite_cascade(sr[:], sz[:], sq[:],
                                                 CW1, CW2, CW3)
                    # custom DVE ops silently no-op on partition-offset APs:
                    # do both wraps full-tile, slice only in the ACT reads.
                    nc.vector.add_range_wrap(sz[:], sr[:], 0.0, PI_F,
                                             TWO_PI_F)
                    if any(f == COS for f, _, _ in segs[l][m]):
                        nc.vector.add_range_wrap(sq[:], sr[:], HALF_PI, PI_F,
                                                 TWO_PI_F)
                    rtiles[m] = (sz, sq)

                # Single descending pass per chunk (chunks ascending):
                # with sin/cos sorted to the TOP partitions and gauss at the
                # bottom, emission order doubles as the table-set grouping
                # ([exp-set funcs + gauss] low chunks, [trig sin/cos] last
                # chunk) -> 2 ACT table loads per layer.  The add_dep chain
                # pins the ACT stream to this order.
                act_chain = []
                for m in range(4):
                    ps, vraw, tmp = psums[m], vraws[m], tmps[m]
                    for fid, slo, shi in reversed(segs[l][m]):
                      for lo, hi in _pieces(slo, shi):
                        s = np.s_[lo:hi, :]
                        if fid == GAUSS:
                            act_chain.append(nc.scalar.activation(
                                tmp[s], ps[s], Act.Square, scale=-W))
                            act_chain.append(nc.scalar.activation(
                                vraw[s], tmp[s], Act.Exp, scale=-1.0))
                        elif fid in (SIN, COS):
                            wsin, wcos = rtiles[m]
                            src_t = wsin if fid == SIN else wcos
                            act_chain.append(nc.scalar.activation(
                                vraw[s], src_t[s], Act.Sin, scale=1.0))
                        elif fid == TANH:
                            act_chain.append(nc.scalar.activation(
                                vraw[s], ps[s], Act.Tanh, scale=-W))
                        elif fid == SIGMOID:
                            act_chain.append(nc.scalar.activation(
                                tmp[s], ps[s], Act.Tanh, scale=-W / 2))
                            nc.vector.tensor_scalar(vraw[s], tmp[s], 0.5, 0.5,
                                                    Alu.mult, Alu.add)
                        elif fid == STEP:
                            # step(S_true) = +1 iff S_psum <= 0
                            nc.vector.tensor_scalar(tmp[s], ps[s], 0.0, None,
                                                    Alu.is_le)
                            nc.vector.tensor_scalar(vraw[s], tmp[s], 2.0, 1.0,
                                                    Alu.mult, Alu.subtract)
                        elif fid == ABS:
                            act_chain.append(nc.scalar.activation(
                                vraw[s], ps[s], Act.Abs, scale=-W))
                        elif fid == INVERT:
                            nc.vector.tensor_scalar(vraw[s], ps[s], W, None,
                                                    Alu.mult)
                        elif fid == LINEAR:
                            nc.vector.tensor_scalar(vraw[s], ps[s], -W, None,
                                                    Alu.mult)
                        elif fid == RELU:
                            act_chain.append(nc.scalar.activation(
                                vraw[s], ps[s], Act.Relu, scale=-W))
                        else:
                            raise ValueError(fid)

                if ACT_CHAIN:
                    # add_dep_helper(x, y) == "x waits on y"
                    for a, b in zip(act_chain, act_chain[1:]):
                        add_dep_helper(b.ins, a.ins, sync=False,
                                       reason="act table order")

                for m in range(4):
                    ps, vraw, tmp = psums[m], vraws[m], tmps[m]
                    if l < L - 1:
                        vt = vpool.tile([128, BS], dt.float32r,
                                        name=f"v{4 + 4 * l + m}")
                        if fast_chain:
                            nc.vector._custom_dve(
                                quirk_op, out=vt[:], in0=vraw[:],
                                s0=1.0, s1=-1.0)
                        else:
                            m1c = rpool.tile([128, BS], dt.float32, name="m1c")
                            nc.vector.tensor_scalar(m1c[:], vraw[:], 1.0, c,
                                                    Alu.is_equal, Alu.mult)
                            nc.vector.tensor_tensor(tmp[:], m1c[:], vraw[:],
                                                    Alu.add)
                            nc.vector.tensor_scalar(m1c[:], vraw[:], -1.0, c,
                                                    Alu.is_equal, Alu.mult)
                            nc.vector.tensor_tensor(vt[:], m1c[:], tmp[:],
                                                    Alu.subtract)
                        V.append(vt)
                        if DEBUG_DUMP:
                            nc.sync.dma_start(
                                dbg[("vraw", l)][m * 128:(m + 1) * 128, :],
                                vraw[:])
                            nc.sync.dma_start(
                                dbg[("vadj", l)][m * 128:(m + 1) * 128, :],
                                vt[:].bitcast(dt.float32))
                    else:
                        nc.sync.dma_start(out_d[m * 128:(m + 1) * 128, :],
                                          vraw[:])
    nc.compile()
    return nc


_CACHE = {}


def _get_program(segs_key, segs, w):
    key = (segs_key, float(w))
    if key not in _CACHE:
        _CACHE[key] = _build_program(segs, w)
    return _CACHE[key]


def kernel(x, shared_weight, edge_src, edge_dst, act_ids):
    from concourse.bass_utils import run_bass_kernel_spmd

    w = float(np.asarray(shared_weight))
    assert w != 0.0
    a_pack, xin, perms, segs = _preprocess(
        np.asarray(x), w, np.asarray(edge_src), np.asarray(edge_dst),
        np.asarray(act_ids))

    segs_key = tuple(tuple(tuple(r) for r in lm) for lseg in segs for lm in lseg)
    nc = _get_program(segs_key, segs, w)

    in_maps = [
        {"xin": np.ascontiguousarray(xin[:, cid * BS:(cid + 1) * BS]),
         "amat": a_pack}
        for cid in range(N_CORES)
    ]
    res = run_bass_kernel_spmd(nc, in_maps, core_ids=list(range(N_CORES)))
    out_sorted = np.concatenate([res.results[cid]["out"]
                                 for cid in range(N_CORES)], axis=1)
    out = np.empty_like(out_sorted)
    out[perms[L - 1]] = out_sorted
    return out.astype(np.float32)



# revision 15
# speedup vs baseline: 1.3116x; 1.3116x over previous
"""Trainium2 Bass kernel for nn_CustomModel_12953621365157 (gnn_message_passing).

Strategy
--------
Data-parallel over the batch axis: 8 cores, 512 batch columns each.

Per layer the reference does gather(edge_src) -> 0.5-weight-with-|g|==1-quirk
-> segment_sum(edge_dst) -> per-node activation.  Because the quirk depends
only on the gathered *value*, it folds into the source node:

    v_adj[n] = v[n] + c*(v[n]==1) - c*(v[n]==-1),   c = (1-w)/w
    summed   = w * (A_l @ v_adj)                    A_l[p,n] = #edges n->p

so each layer is a dense [P x N_l] x [N_l x B_s] matmul on the PE.  A_l is
built host-side from the edge lists; counts are small integers so A is
stored bf16 (exact), halving HBM traffic vs fp32.  V tiles are bf16: the
quirk fold runs in fp32 (preserving reference semantics exactly) and only
the final store rounds.  PSUM accumulation stays fp32.

Activations: nodes are pre-sorted by activation id, grouped by the engine
that writes the result tile, so dispatch is a few partition-range
instructions spread over three engines:
  ACT  (single table set exp_and_others, zero swaps): gauss-exp, tanh,
       abs, step (Sign with +eps bias), sigmoid-tanh
  DVE  : gauss-square (stt), trig range reduction (round-to-nearest
       magic-number trick) + odd deg-7 sin polynomial custom ops,
       quirk fold (fp32 compare, bf16 store)
  Pool : sigmoid finish, relu, linear, invert, sin/cos final multiply
Avoiding the Sin ACT table removes all 8 LoadActFuncSet swaps and the
serializing act-order chain of the previous version.

Chunks are processed m=3..0; the next layer's accumulation consumes the
new V tiles in production order so the last-produced tile is needed last,
hiding act->quirk latency behind the first matmuls of the next layer.
"""

import numpy as np
import ml_dtypes

N_IN = 512
P = 512
L = 4
E = 32768
B = 4096
N_CORES = 8
BS = B // N_CORES  # 512 batch columns per core

# activation ids (order matches reference activations_dict)
LINEAR, STEP, SIN, COS, GAUSS, TANH, SIGMOID, ABS, INVERT, RELU = range(10)
# node sort order per layer, grouped by the engine that writes vraw:
# ACT-written funcs at the bottom, Pool-written above, trig on top, so a
# single cross-engine overlap boundary exists per chunk.
FUNC_ORDER = [GAUSS, TANH, ABS, STEP, SIGMOID, RELU, LINEAR, INVERT, SIN, COS]
ORDER_RANK = {f: i for i, f in enumerate(FUNC_ORDER)}

NKS = [4, 8, 12, 16]                      # src tiles per layer
PANEL_OFF = []                            # column offset of panel (l, m)
_off = 0
for _l in range(L):
    for _m in range(4):
        PANEL_OFF.append(_off)
        _off += NKS[_l] * 128
TOTCOL = _off                             # 20480

# V-tile order: k<4 input chunks (natural); k>=4: lp=(k-4)//4, i=(k-4)%4
# -> chunk 3-i of layer lp (chunks are produced m=3..0).


def _src_rowbase(k):
    if k < 4:
        return k * 128
    lp, i = (k - 4) // 4, (k - 4) % 4
    return N_IN + lp * P + (3 - i) * 128


def _preprocess(x, w, edge_src, edge_dst, act_ids):
    """Host-side: node sort per layer, packed bf16 A panels, quirk fold."""
    c = (1.0 - w) / w

    perms = []      # perms[l][p_sorted] = orig node j
    inv_perms = []  # inv_perms[l][orig j] = p_sorted
    segs = []       # segs[l][m] = list of (func_id, lo, hi) within chunk m
    for l in range(L):
        ids = np.asarray(act_ids[l])
        key = np.array([ORDER_RANK[int(i)] for i in ids])
        perm = np.argsort(key, kind="stable")
        inv = np.empty(P, np.int64)
        inv[perm] = np.arange(P)
        perms.append(perm)
        inv_perms.append(inv)
        ids_sorted = ids[perm]
        layer_segs = []
        for m in range(4):
            chunk = ids_sorted[m * 128:(m + 1) * 128]
            runs = []
            lo = 0
            for i in range(1, 129):
                if i == 128 or chunk[i] != chunk[lo]:
                    runs.append((int(chunk[lo]), lo, i))
                    lo = i
            layer_segs.append(runs)
        segs.append(layer_segs)

    inv_stack = np.stack(inv_perms)  # [L, P]

    # dense per-layer adjacency in sorted coordinates
    a_dense = [np.zeros((N_IN + l * P, P), np.float32) for l in range(L)]
    for l in range(L):
        src = np.asarray(edge_src[l]).astype(np.int64)
        dst = np.asarray(edge_dst[l]).astype(np.int64)
        g = src.copy()
        m = g >= N_IN
        lp = (g[m] - N_IN) // P
        j = (g[m] - N_IN) % P
        g[m] = N_IN + lp * P + inv_stack[lp, j]
        d = inv_perms[l][dst]
        np.add.at(a_dense[l], (g, d), 1.0)

    # pack into [128, TOTCOL]: panel (l, m) col k*128 + j, row p =
    # A_l[src_rowbase(k) + p, m*128 + j]  (k in V-tile order)
    a_pack = np.zeros((128, TOTCOL), np.float32)
    for l in range(L):
        nk = NKS[l]
        for m in range(4):
            off = PANEL_OFF[l * 4 + m]
            for k in range(nk):
                rb = _src_rowbase(k)
                blk = a_dense[l][rb:rb + 128, m * 128:(m + 1) * 128]
                a_pack[:, off + k * 128: off + (k + 1) * 128] = blk
    assert a_pack.max() < 128  # exact in bf16

    xa = x.astype(np.float32)
    if c != 0.0:
        xa = xa + c * (xa == 1.0) - c * (xa == -1.0)
    xin = -xa  # V tiles hold -v_adj
    # pre-swizzled: xpack[cid][p, k*BS + j] = xin[k*128 + p, cid*BS + j]
    xpacks = []
    for cid in range(N_CORES):
        sl = xin[:, cid * BS:(cid + 1) * BS]             # [512, BS]
        xp = sl.reshape(4, 128, BS).transpose(1, 0, 2).reshape(128, 4 * BS)
        xpacks.append(np.ascontiguousarray(xp.astype(ml_dtypes.bfloat16)))
    return a_pack.astype(ml_dtypes.bfloat16), xpacks, perms, segs


# odd deg-7 sin minimax coefficients on [-pi-eps, pi+eps]:
# sin(x) ~= x * (S0 + S1 t + S2 t^2 + S3 t^3),  t = x^2
SINP = [9.99876641e-01, -1.66216805e-01, 8.08060368e-03, -1.52742172e-04]
# even deg-6 cos fit on the same range: cos(x) ~= C0 + C1 t + C2 t^2 + C3 t^3
COSP = [9.98937591e-01, -4.96113910e-01, 3.94725721e-02, -9.88522393e-04]

_OPS = None


def _get_custom_ops():
    """Custom DVE ops: quirk fold, trig range reduction, sin polynomial."""
    global _OPS
    if _OPS is not None:
        return _OPS
    import concourse.dve_ops as dve_ops
    from concourse.dve_spec import (Spec, Src0, C0, C1, C2, C3, lower,
                                    _has_src1, _spill_c3_to_src1, Bin)
    from concourse.dve_uop import AluOp, DveOpSpec

    def eq(a, b):
        return Bin(AluOp.IS_EQ, a, b)

    def _register(name, spec):
        if name not in dve_ops._SUB_OPCODE_FOR_NAME:
            row = max(dve_ops._SUB_OPCODE_FOR_NAME.values()) + 1
            assert row < 0x20
            dve_ops._SUB_OPCODE_FOR_NAME[name] = row
        opcode = dve_ops._SUB_OPCODE_FOR_NAME[name]
        shas = {}
        for ver in ("v3", "v4"):
            u = lower(spec, ver=ver)
            shas[ver] = DveOpSpec(name=name, opcode=opcode, uops=u,
                                  rd1_en=_has_src1(spec)).sha(ver)
        op = dve_ops.DveOp(name, spec, subdim=False, uops_sha=shas)
        for i, o in enumerate(dve_ops.OPS):
            if o.name == name:
                dve_ops.OPS[i] = op
                break
        else:
            dve_ops.OPS.append(op)
        dve_ops.CUSTOM_DVE_SPECS[name] = spec
        return op

    # quirk fold: out = -v_adj = c*(v==-1) - (c*(v==1) + v)
    # s0=-1.0, s1=1.0, imm2=c.  bf16 out AP: the fp32 compare runs before
    # the store rounds, preserving reference semantics exactly.
    quirk = _register("QF_G_ANT", Spec(
        body=eq(Src0, C0) * C2 - (eq(Src0, C1) * C2 + Src0),
        reference=lambda in0, in1, s0, s1, imm2: (
            (in0 == np.float32(s0)).astype(np.float32) * np.float32(imm2)
            - ((in0 == np.float32(s1)).astype(np.float32) * np.float32(imm2)
               + in0)),
    ))

    # trig range reduction: q = s0*ps; out = (q - rne(q)) * imm2
    # rne via the magic-number trick (s1 = 1.5*2^23), valid for |q| < 2^22.
    def _red_ref(in0, in1, s0, s1, imm2):
        q = (in0 * np.float32(s0)).astype(np.float32)
        k = ((q + np.float32(s1)).astype(np.float32)
             - np.float32(s1)).astype(np.float32)
        return ((q - k).astype(np.float32) * np.float32(imm2)).astype(np.float32)

    _q = C0 * Src0
    red = _register("TRIGRED_ANT", Spec(
        body=(_q - ((_q + C1) - C1)) * C2,
        reference=_red_ref,
    ))

    # u = ((S3*t + S2)*t + S1)*t + S0, t = x^2; sin(x) ~= u*x.
    # S3 rides C3 -> spilled to in1 (pass a [128,1] tile holding SINP[3]).
    def _sinu_ref(in0, in1, s0, s1, imm2):
        t = (in0 * in0).astype(np.float32)
        u = np.broadcast_to(np.float32(SINP[3]), in0.shape).astype(np.float32)
        for cc in (imm2, s1, s0):
            u = (u * t + np.float32(cc)).astype(np.float32)
        return u

    t = Src0 * Src0
    sinu = _register("SINU_ANT", Spec(
        body=_spill_c3_to_src1(((C3 * t + C2) * t + C1) * t + C0),
        reference=_sinu_ref,
    ))

    # cos(x) ~= ((P3*t + P2)*t + P1)*t + P0, t = x^2 (direct value, no
    # final multiply).  P3 rides C3 -> in1 = [128,1] tile of COSP[3].
    def _cosq_ref(in0, in1, s0, s1, imm2):
        t = (in0 * in0).astype(np.float32)
        u = np.broadcast_to(np.float32(COSP[3]), in0.shape).astype(np.float32)
        for cc in (imm2, s1, s0):
            u = (u * t + np.float32(cc)).astype(np.float32)
        return u

    cosq = _register("COSQ_ANT", Spec(
        body=_spill_c3_to_src1(((C3 * t + C2) * t + C1) * t + C0),
        reference=_cosq_ref,
    ))

    _OPS = (quirk, red, sinu, cosq)
    return _OPS


def _build_program(segs, w):
    import concourse.bacc as bacc
    import concourse.mybir as mybir
    import concourse.tile as tile

    quirk_op, red_op, sinu_op, cosq_op = _get_custom_ops()

    dt = mybir.dt
    Act = mybir.ActivationFunctionType
    Alu = mybir.AluOpType
    W = float(w)
    assert W > 0.0
    c = (1.0 - W) / W

    TWO_PI_F = float(np.float32(2.0 * np.pi))
    INV_2PI = float(np.float32(1.0 / (2.0 * np.pi)))
    PI_F = float(np.float32(np.pi))
    HALF_PI = float(np.float32(np.pi / 2))
    MAGIC = float(np.float32(1.5 * 2 ** 23))

    nc = bacc.Bacc("TRN2", target_bir_lowering=False, debug=False,
                   num_devices=N_CORES)
    xin = nc.dram_tensor("xin", [128, 4 * BS], dt.bfloat16,
                         kind="ExternalInput").ap()
    a_d = nc.dram_tensor("amat", [128, TOTCOL], dt.bfloat16,
                         kind="ExternalInput").ap()
    out_d = nc.dram_tensor("out", [P, BS], dt.float32,
                           kind="ExternalOutput").ap()

    def _pieces(lo, hi):
        p = (lo // 32) * 32
        out = []
        while p < hi:
            end = min(hi, 64) if p == 32 else hi
            out.append((p, end))
            p = end
        return out

    with tile.TileContext(nc) as tc:
        with tc.tile_pool(name="Ap", bufs=1) as apool, \
             tc.tile_pool(name="Vp", bufs=1) as vpool, \
             tc.tile_pool(name="raw", bufs=8) as rpool, \
             tc.tile_pool(name="scr", bufs=3) as spool, \
             tc.tile_pool(name="ps", bufs=8, space="PSUM") as ppool:

            # poly 4th coefficient columns (C3 spill for SINU/COSQ)
            sincol = vpool.tile([128, 1], dt.float32, name="sincol")
            nc.vector.memset(sincol[:], SINP[3])
            coscol = vpool.tile([128, 1], dt.float32, name="coscol")
            nc.vector.memset(coscol[:], COSP[3])
            # +eps bias column for STEP's Sign (resolves summed==0 to +1)
            epscol = vpool.tile([128, 1], dt.float32, name="epscol")
            nc.vector.memset(epscol[:], 1e-30)

            # input node values (already quirk-folded & negated on host)
            xt = vpool.tile([128, 4 * BS], dt.bfloat16, name="xt")
            V = [xt[:, k * BS:(k + 1) * BS] for k in range(4)]

            panels = {}
            nc.sync.dma_start(xt[:], xin[:, :])
            for l in range(L):
                for m in (3, 2, 1, 0):
                    pt = apool.tile([128, NKS[l] * 128], dt.bfloat16,
                                    name=f"a{l}_{m}")
                    panels[(l, m)] = pt
                    off = PANEL_OFF[l * 4 + m]
                    nc.sync.dma_start(pt[:], a_d[:, off:off + NKS[l] * 128])

            for l in range(L):
                nk = NKS[l]
                new_v = {}
                for m in (3, 2, 1, 0):
                    pt = panels[(l, m)]
                    ps = ppool.tile([128, BS], dt.float32, name="ps")
                    for k in range(nk):
                        nc.tensor.matmul(
                            ps[:], pt[:, k * 128:(k + 1) * 128],
                            V[k], start=(k == 0), stop=(k == nk - 1))

                    sl = segs[l][m]
                    has_sin = any(f == SIN for f, _, _ in sl)
                    has_cos = any(f == COS for f, _, _ in sl)
                    has_gauss = any(f == GAUSS for f, _, _ in sl)
                    has_sig = any(f == SIGMOID for f, _, _ in sl)

                    vraw = rpool.tile([128, BS], dt.float32, name="vraw")
                    rt = yt = ut = uct = tmp = tmp2 = None
                    if has_sin or has_cos:
                        rt = spool.tile([128, BS], dt.float32, name="rt")
                        nc.vector._custom_dve(red_op, out=rt[:], in0=ps[:],
                                              s0=-W * INV_2PI, s1=MAGIC,
                                              imm2=TWO_PI_F)
                    if has_sin:
                        ut = spool.tile([128, BS], dt.float32, name="ut")
                        nc.vector._custom_dve(sinu_op, out=ut[:], in0=rt[:],
                                              in1=sincol[:], s0=SINP[0],
                                              s1=SINP[1], imm2=SINP[2])
                    if has_cos:
                        # cos(summed) = cosq(r) directly (even poly)
                        uct = spool.tile([128, BS], dt.float32, name="uct")
                        nc.vector._custom_dve(cosq_op, out=uct[:], in0=rt[:],
                                              in1=coscol[:], s0=COSP[0],
                                              s1=COSP[1], imm2=COSP[2])
                    if has_gauss:
                        # tmp = summed^2 (ACT Square with scale)
                        tmp = spool.tile([128, BS], dt.float32, name="tmp")
                        nc.scalar.activation(tmp[:], ps[:], Act.Square,
                                             scale=-W)
                    if has_sig:
                        # ACT part of sigmoid, full tile (extra rows unread)
                        tmp2 = spool.tile([128, BS], dt.float32, name="tmp2")
                        nc.scalar.activation(tmp2[:], ps[:], Act.Tanh,
                                             scale=-W / 2)

                    # vraw pieces, descending partition order (true owner of
                    # any 32-alignment overlap writes last in program order)
                    for fid, slo, shi in reversed(sl):
                        for lo, hi in _pieces(slo, shi):
                            s = np.s_[lo:hi, :]
                            if fid == COS:
                                nc.gpsimd.tensor_scalar(
                                    vraw[s], uct[s], 1.0, None, Alu.mult)
                            elif fid == SIN:
                                nc.gpsimd.tensor_tensor(
                                    vraw[s], ut[s], rt[s], Alu.mult)
                            elif fid == INVERT:
                                # Pool cannot read PSUM -> DVE
                                nc.vector.tensor_scalar(
                                    vraw[s], ps[s], W, None, Alu.mult)
                            elif fid == LINEAR:
                                nc.vector.tensor_scalar(
                                    vraw[s], ps[s], -W, None, Alu.mult)
                            elif fid == RELU:
                                nc.scalar.activation(
                                    vraw[s], ps[s], Act.Relu, scale=-W)
                            elif fid == SIGMOID:
                                nc.gpsimd.tensor_scalar(
                                    vraw[s], tmp2[s], 0.5, 0.5, Alu.mult,
                                    Alu.add)
                            elif fid == STEP:
                                # +1 iff summed >= 0 iff -ps >= 0; +eps bias
                                # resolves summed==0 to +1 as the ref does
                                nc.scalar.activation(
                                    vraw[s], ps[s], Act.Sign, scale=-1.0,
                                    bias=epscol[lo:hi])
                            elif fid == ABS:
                                nc.scalar.activation(
                                    vraw[s], ps[s], Act.Abs, scale=-W)
                            elif fid == TANH:
                                nc.scalar.activation(
                                    vraw[s], ps[s], Act.Tanh, scale=-W)
                            elif fid == GAUSS:
                                nc.scalar.activation(
                                    vraw[s], tmp[s], Act.Exp, scale=-1.0)
                            else:
                                raise ValueError(fid)

                    if l < L - 1:
                        vt = vpool.tile([128, BS], dt.bfloat16,
                                        name=f"v{4 + 4 * l + (3 - m)}")
                        nc.vector._custom_dve(quirk_op, out=vt[:],
                                              in0=vraw[:], s0=-1.0, s1=1.0,
                                              imm2=c)
                        new_v[m] = vt[:]
                    else:
                        nc.sync.dma_start(out_d[m * 128:(m + 1) * 128, :],
                                          vraw[:])
                if l < L - 1:
                    # next layer consumes new tiles in production order
                    # (chunk 3 first), matching _src_rowbase
                    V.extend(new_v[m] for m in (3, 2, 1, 0))
    nc.compile()
    return nc


_CACHE = {}


def _get_program(segs_key, segs, w):
    key = (segs_key, float(w))
    if key not in _CACHE:
        _CACHE[key] = _build_program(segs, w)
    return _CACHE[key]


def kernel(x, shared_weight, edge_src, edge_dst, act_ids):
    from concourse.bass_utils import run_bass_kernel_spmd

    w = float(np.asarray(shared_weight))
    assert w > 0.0
    a_pack, xpacks, perms, segs = _preprocess(
        np.asarray(x), w, np.asarray(edge_src), np.asarray(edge_dst),
        np.asarray(act_ids))

    segs_key = tuple(tuple(tuple(r) for r in lm) for lseg in segs for lm in lseg)
    nc = _get_program(segs_key, segs, w)

    in_maps = [
        {"xin": xpacks[cid], "amat": a_pack}
        for cid in range(N_CORES)
    ]
    res = run_bass_kernel_spmd(nc, in_maps, core_ids=list(range(N_CORES)))
    out_sorted = np.concatenate([res.results[cid]["out"]
                                 for cid in range(N_CORES)], axis=1)
    out = np.empty_like(out_sorted)
    out[perms[L - 1]] = out_sorted
    return out.astype(np.float32)


# ---------------------------------------------------------------------------
# Host-side numpy emulation of the device program, for fast numerics checks
# (python kernel_selftest) without touching hardware.
def _emulate(x, shared_weight, edge_src, edge_dst, act_ids):
    w = float(np.asarray(shared_weight))
    a_pack, xpacks, perms, segs = _preprocess(
        np.asarray(x), w, np.asarray(edge_src), np.asarray(edge_dst),
        np.asarray(act_ids))
    c = np.float32((1.0 - w) / w)
    W = np.float32(w)
    bf = ml_dtypes.bfloat16
    outs = []
    for cid in range(N_CORES):
        xp = xpacks[cid]
        V = [xp[:, k * BS:(k + 1) * BS] for k in range(4)]
        vraw_last = {}
        for l in range(L):
            nk = NKS[l]
            new_v = {}
            for m in (3, 2, 1, 0):
                off = PANEL_OFF[l * 4 + m]
                ps = np.zeros((128, BS), np.float32)
                for k in range(nk):
                    A = a_pack[:, off + k * 128: off + (k + 1) * 128]
                    ps += A.astype(np.float32).T @ V[k].astype(np.float32)
                vraw = np.zeros((128, BS), np.float32)
                summed = (-W * ps).astype(np.float32)
                # trig
                q = (ps * np.float32(-w / (2 * np.pi))).astype(np.float32)
                k2 = ((q + np.float32(1.5 * 2**23)).astype(np.float32)
                      - np.float32(1.5 * 2**23)).astype(np.float32)
                r = ((q - k2) * np.float32(2 * np.pi)).astype(np.float32)
                def sinpoly(xx):
                    t = (xx * xx).astype(np.float32)
                    u = np.broadcast_to(np.float32(SINP[3]), xx.shape)
                    for cc in (SINP[2], SINP[1], SINP[0]):
                        u = (u * t + np.float32(cc)).astype(np.float32)
                    return u

                def cospoly(xx):
                    t = (xx * xx).astype(np.float32)
                    u = np.broadcast_to(np.float32(COSP[3]), xx.shape)
                    for cc in (COSP[2], COSP[1], COSP[0]):
                        u = (u * t + np.float32(cc)).astype(np.float32)
                    return u
                for fid, lo, hi in segs[l][m]:
                    s = np.s_[lo:hi]
                    if fid == LINEAR:
                        vraw[s] = summed[s]
                    elif fid == INVERT:
                        vraw[s] = -summed[s]
                    elif fid == RELU:
                        vraw[s] = np.maximum(summed[s], 0)
                    elif fid == STEP:
                        vraw[s] = np.where(-ps[s] + np.float32(1e-30) >= 0,
                                           1.0, -1.0).astype(np.float32)
                    elif fid == ABS:
                        vraw[s] = np.abs(summed[s])
                    elif fid == TANH:
                        vraw[s] = np.tanh(summed[s]).astype(np.float32)
                    elif fid == SIGMOID:
                        t2 = np.tanh(summed[s] / 2).astype(np.float32)
                        vraw[s] = (t2 * np.float32(0.5)
                                   + np.float32(0.5)).astype(np.float32)
                    elif fid == GAUSS:
                        t2 = ((ps[s] * W * W) * ps[s]).astype(np.float32)
                        vraw[s] = np.exp(-t2).astype(np.float32)
                    elif fid == SIN:
                        vraw[s] = (sinpoly(r[s]) * r[s]).astype(np.float32)
                    elif fid == COS:
                        vraw[s] = cospoly(r[s])
                if l < L - 1:
                    va = ((vraw == -1).astype(np.float32) * c
                          - ((vraw == 1).astype(np.float32) * c + vraw))
                    new_v[m] = va.astype(bf)
                else:
                    vraw_last[m] = vraw
            if l < L - 1:
                V.extend(new_v[m] for m in (3, 2, 1, 0))
        outs.append(np.concatenate([vraw_last[m] for m in range(4)], 0))
    out_sorted = np.concatenate(outs, axis=1)
    out = np.empty_like(out_sorted)
    out[perms[L - 1]] = out_sorted
    return out.astype(np.float32)


# revision 21
# speedup vs baseline: 1.3702x; 1.0447x over previous
"""Trainium2 Bass kernel for nn_CustomModel_12953621365157 (gnn_message_passing).

Strategy
--------
Data-parallel over the batch axis: 8 cores, 512 batch columns each.

Per layer the reference does gather(edge_src) -> 0.5-weight-with-|g|==1-quirk
-> segment_sum(edge_dst) -> per-node activation.  Because the quirk depends
only on the gathered *value*, it folds into the source node:

    v_adj[n] = v[n] + c*(v[n]==1) - c*(v[n]==-1),   c = (1-w)/w
    summed   = w * (A_l @ v_adj)                    A_l[p,n] = #edges n->p

so each layer is a dense [P x N_l] x [N_l x B_s] matmul on the PE.  A_l is
built host-side from the edge lists; counts are small integers so A is
stored bf16 (exact), halving HBM traffic vs fp32.  V tiles are bf16: the
quirk fold runs in fp32 (preserving reference semantics exactly) and only
the final store rounds.  PSUM accumulation stays fp32.

Activations: nodes are pre-sorted by activation id, grouped by the engine
that writes the result tile, so dispatch is a few partition-range
instructions spread over three engines:
  ACT  (single table set exp_and_others, zero swaps): gauss-exp, tanh,
       abs, step (Sign with +eps bias), sigmoid-tanh
  DVE  : gauss-square (stt), trig range reduction (round-to-nearest
       magic-number trick) + odd deg-7 sin polynomial custom ops,
       quirk fold (fp32 compare, bf16 store)
  Pool : sigmoid finish, relu, linear, invert, sin/cos final multiply
Avoiding the Sin ACT table removes all 8 LoadActFuncSet swaps and the
serializing act-order chain of the previous version.

Chunks are processed m=3..0; the next layer's accumulation consumes the
new V tiles in production order so the last-produced tile is needed last,
hiding act->quirk latency behind the first matmuls of the next layer.
"""

import numpy as np
import ml_dtypes

N_IN = 512
P = 512
L = 4
E = 32768
B = 4096
N_CORES = 8
BS = B // N_CORES  # 512 batch columns per core

# activation ids (order matches reference activations_dict)
LINEAR, STEP, SIN, COS, GAUSS, TANH, SIGMOID, ABS, INVERT, RELU = range(10)
# node sort order per layer: expensive multi-op funcs (trig chain, gauss,
# sigmoid) in the low chunks, which are processed first each layer so their
# long act pipelines overlap the layer's remaining matmuls; cheap one-op
# funcs (relu/linear/invert) on top so the per-layer tail is short.
FUNC_ORDER = [SIN, COS, GAUSS, SIGMOID, TANH, ABS, STEP, RELU, LINEAR, INVERT]
ORDER_RANK = {f: i for i, f in enumerate(FUNC_ORDER)}

NKS = [4, 8, 12, 16]                      # src tiles per layer
PANEL_OFF = []                            # column offset of panel (l, m)
_off = 0
for _l in range(L):
    for _m in range(4):
        PANEL_OFF.append(_off)
        _off += NKS[_l] * 128
TOTCOL = _off                             # 20480

# V-tile order: k<4 input chunks; k>=4: chunk (k-4)%4 of layer (k-4)//4
# (chunks are produced m=0..3).


def _src_rowbase(k):
    if k < 4:
        return k * 128
    lp, i = (k - 4) // 4, (k - 4) % 4
    return N_IN + lp * P + i * 128


def _preprocess(x, w, edge_src, edge_dst, act_ids):
    """Host-side: node sort per layer, packed bf16 A panels, quirk fold."""
    c = (1.0 - w) / w

    perms = []      # perms[l][p_sorted] = orig node j
    inv_perms = []  # inv_perms[l][orig j] = p_sorted
    segs = []       # segs[l][m] = list of (func_id, lo, hi) within chunk m
    for l in range(L):
        ids = np.asarray(act_ids[l])
        key = np.array([ORDER_RANK[int(i)] for i in ids])
        perm = np.argsort(key, kind="stable")
        inv = np.empty(P, np.int64)
        inv[perm] = np.arange(P)
        perms.append(perm)
        inv_perms.append(inv)
        ids_sorted = ids[perm]
        layer_segs = []
        for m in range(4):
            chunk = ids_sorted[m * 128:(m + 1) * 128]
            runs = []
            lo = 0
            for i in range(1, 129):
                if i == 128 or chunk[i] != chunk[lo]:
                    runs.append((int(chunk[lo]), lo, i))
                    lo = i
            layer_segs.append(runs)
        segs.append(layer_segs)

    inv_stack = np.stack(inv_perms)  # [L, P]

    # dense per-layer adjacency in sorted coordinates
    a_dense = [np.zeros((N_IN + l * P, P), np.float32) for l in range(L)]
    for l in range(L):
        src = np.asarray(edge_src[l]).astype(np.int64)
        dst = np.asarray(edge_dst[l]).astype(np.int64)
        g = src.copy()
        m = g >= N_IN
        lp = (g[m] - N_IN) // P
        j = (g[m] - N_IN) % P
        g[m] = N_IN + lp * P + inv_stack[lp, j]
        d = inv_perms[l][dst]
        np.add.at(a_dense[l], (g, d), 1.0)

    # pack into [128, TOTCOL]: panel (l, m) col k*128 + j, row p =
    # A_l[src_rowbase(k) + p, m*128 + j]  (k in V-tile order)
    a_pack = np.zeros((128, TOTCOL), np.float32)
    for l in range(L):
        nk = NKS[l]
        for m in range(4):
            off = PANEL_OFF[l * 4 + m]
            for k in range(nk):
                rb = _src_rowbase(k)
                blk = a_dense[l][rb:rb + 128, m * 128:(m + 1) * 128]
                a_pack[:, off + k * 128: off + (k + 1) * 128] = blk
    assert a_pack.max() < 128  # exact in bf16

    xa = x.astype(np.float32)
    if c != 0.0:
        xa = xa + c * (xa == 1.0) - c * (xa == -1.0)
    xin = -xa  # V tiles hold -v_adj
    # pre-swizzled: xpack[cid][p, k*BS + j] = xin[k*128 + p, cid*BS + j]
    xpacks = []
    for cid in range(N_CORES):
        sl = xin[:, cid * BS:(cid + 1) * BS]             # [512, BS]
        xp = sl.reshape(4, 128, BS).transpose(1, 0, 2).reshape(128, 4 * BS)
        xpacks.append(np.ascontiguousarray(xp.astype(ml_dtypes.bfloat16)))
    return a_pack.astype(ml_dtypes.bfloat16), xpacks, perms, segs


# odd deg-7 sin minimax coefficients on [-pi-eps, pi+eps]:
# sin(x) ~= x * (S0 + S1 t + S2 t^2 + S3 t^3),  t = x^2
SINP = [9.99876641e-01, -1.66216805e-01, 8.08060368e-03, -1.52742172e-04]
# even deg-6 cos fit on the same range: cos(x) ~= C0 + C1 t + C2 t^2 + C3 t^3
COSP = [9.98937591e-01, -4.96113910e-01, 3.94725721e-02, -9.88522393e-04]

_OPS = None


def _get_custom_ops():
    """Custom DVE ops: quirk fold, trig range reduction, sin polynomial."""
    global _OPS
    if _OPS is not None:
        return _OPS
    import concourse.dve_ops as dve_ops
    from concourse.dve_spec import (Spec, Src0, C0, C1, C2, C3, lower,
                                    _has_src1, _spill_c3_to_src1, Bin)
    from concourse.dve_uop import AluOp, DveOpSpec

    def eq(a, b):
        return Bin(AluOp.IS_EQ, a, b)

    def _register(name, spec):
        if name not in dve_ops._SUB_OPCODE_FOR_NAME:
            row = max(dve_ops._SUB_OPCODE_FOR_NAME.values()) + 1
            assert row < 0x20
            dve_ops._SUB_OPCODE_FOR_NAME[name] = row
        opcode = dve_ops._SUB_OPCODE_FOR_NAME[name]
        shas = {}
        for ver in ("v3", "v4"):
            u = lower(spec, ver=ver)
            shas[ver] = DveOpSpec(name=name, opcode=opcode, uops=u,
                                  rd1_en=_has_src1(spec)).sha(ver)
        op = dve_ops.DveOp(name, spec, subdim=False, uops_sha=shas)
        for i, o in enumerate(dve_ops.OPS):
            if o.name == name:
                dve_ops.OPS[i] = op
                break
        else:
            dve_ops.OPS.append(op)
        dve_ops.CUSTOM_DVE_SPECS[name] = spec
        return op

    # quirk fold: out = -v_adj = c*(v==-1) - (c*(v==1) + v)
    # s0=-1.0, s1=1.0, imm2=c.  bf16 out AP: the fp32 compare runs before
    # the store rounds, preserving reference semantics exactly.
    quirk = _register("QF_G_ANT", Spec(
        body=eq(Src0, C0) * C2 - (eq(Src0, C1) * C2 + Src0),
        reference=lambda in0, in1, s0, s1, imm2: (
            (in0 == np.float32(s0)).astype(np.float32) * np.float32(imm2)
            - ((in0 == np.float32(s1)).astype(np.float32) * np.float32(imm2)
               + in0)),
    ))

    # trig range reduction: q = s0*ps; out = (q - rne(q)) * imm2
    # rne via the magic-number trick (s1 = 1.5*2^23), valid for |q| < 2^22.
    def _red_ref(in0, in1, s0, s1, imm2):
        q = (in0 * np.float32(s0)).astype(np.float32)
        k = ((q + np.float32(s1)).astype(np.float32)
             - np.float32(s1)).astype(np.float32)
        return ((q - k).astype(np.float32) * np.float32(imm2)).astype(np.float32)

    _q = C0 * Src0
    red = _register("TRIGRED_ANT", Spec(
        body=(_q - ((_q + C1) - C1)) * C2,
        reference=_red_ref,
    ))

    # u = ((S3*t + S2)*t + S1)*t + S0, t = x^2; sin(x) ~= u*x.
    # S3 rides C3 -> spilled to in1 (pass a [128,1] tile holding SINP[3]).
    def _sinu_ref(in0, in1, s0, s1, imm2):
        t = (in0 * in0).astype(np.float32)
        u = np.broadcast_to(np.float32(SINP[3]), in0.shape).astype(np.float32)
        for cc in (imm2, s1, s0):
            u = (u * t + np.float32(cc)).astype(np.float32)
        return u

    t = Src0 * Src0
    sinu = _register("SINU_ANT", Spec(
        body=_spill_c3_to_src1(((C3 * t + C2) * t + C1) * t + C0),
        reference=_sinu_ref,
    ))

    # cos(x) ~= ((P3*t + P2)*t + P1)*t + P0, t = x^2 (direct value, no
    # final multiply).  P3 rides C3 -> in1 = [128,1] tile of COSP[3].
    def _cosq_ref(in0, in1, s0, s1, imm2):
        t = (in0 * in0).astype(np.float32)
        u = np.broadcast_to(np.float32(COSP[3]), in0.shape).astype(np.float32)
        for cc in (imm2, s1, s0):
            u = (u * t + np.float32(cc)).astype(np.float32)
        return u

    cosq = _register("COSQ_ANT", Spec(
        body=_spill_c3_to_src1(((C3 * t + C2) * t + C1) * t + C0),
        reference=_cosq_ref,
    ))

    _OPS = (quirk, red, sinu, cosq)
    return _OPS


def _build_program(segs, w):
    import concourse.bacc as bacc
    import concourse.mybir as mybir
    import concourse.tile as tile

    quirk_op, red_op, sinu_op, cosq_op = _get_custom_ops()

    dt = mybir.dt
    Act = mybir.ActivationFunctionType
    Alu = mybir.AluOpType
    W = float(w)
    assert W > 0.0
    c = (1.0 - W) / W

    TWO_PI_F = float(np.float32(2.0 * np.pi))
    INV_2PI = float(np.float32(1.0 / (2.0 * np.pi)))
    PI_F = float(np.float32(np.pi))
    HALF_PI = float(np.float32(np.pi / 2))
    MAGIC = float(np.float32(1.5 * 2 ** 23))

    nc = bacc.Bacc("TRN2", target_bir_lowering=False, debug=False,
                   num_devices=N_CORES)
    xin = nc.dram_tensor("xin", [128, 4 * BS], dt.bfloat16,
                         kind="ExternalInput").ap()
    a_d = nc.dram_tensor("amat", [128, TOTCOL], dt.bfloat16,
                         kind="ExternalInput").ap()
    out_d = nc.dram_tensor("out", [P, BS], dt.float32,
                           kind="ExternalOutput").ap()

    def _pieces(lo, hi):
        p = (lo // 32) * 32
        out = []
        while p < hi:
            end = min(hi, 64) if p == 32 else hi
            out.append((p, end))
            p = end
        return out

    with tile.TileContext(nc) as tc:
        with tc.tile_pool(name="Ap", bufs=1) as apool, \
             tc.tile_pool(name="Vp", bufs=1) as vpool, \
             tc.tile_pool(name="raw", bufs=8) as rpool, \
             tc.tile_pool(name="scr", bufs=3) as spool, \
             tc.tile_pool(name="ps", bufs=8, space="PSUM") as ppool:

            # poly 4th coefficient columns (C3 spill for SINU/COSQ)
            sincol = vpool.tile([128, 1], dt.float32, name="sincol")
            nc.vector.memset(sincol[:], SINP[3])
            coscol = vpool.tile([128, 1], dt.float32, name="coscol")
            nc.vector.memset(coscol[:], COSP[3])
            # +eps bias column for STEP's Sign (resolves summed==0 to +1)
            epscol = vpool.tile([128, 1], dt.float32, name="epscol")
            nc.vector.memset(epscol[:], 1e-30)

            # input node values (already quirk-folded & negated on host)
            xt = vpool.tile([128, 4 * BS], dt.bfloat16, name="xt")
            V = [xt[:, k * BS:(k + 1) * BS] for k in range(4)]

            panels = {}
            for l in range(L):
                for m in range(4):
                    panels[(l, m)] = apool.tile([128, NKS[l] * 128],
                                                dt.bfloat16, name=f"a{l}_{m}")

            def _panel_dma(l, m):
                off = PANEL_OFF[l * 4 + m]
                nc.sync.dma_start(panels[(l, m)][:],
                                  a_d[:, off:off + NKS[l] * 128])

            # priority order: first chunk's stationary, then the input tiles,
            # then the remaining panels in use order
            _panel_dma(0, 0)
            for k in range(4):
                nc.sync.dma_start(xt[:, k * BS:(k + 1) * BS],
                                  xin[:, k * BS:(k + 1) * BS])
            for l in range(L):
                for m in range(4):
                    if (l, m) != (0, 0):
                        _panel_dma(l, m)

            for l in range(L):
                nk = NKS[l]
                new_v = {}
                for m in range(4):
                    pt = panels[(l, m)]
                    ps = ppool.tile([128, BS], dt.float32, name="ps")
                    for k in range(nk):
                        nc.tensor.matmul(
                            ps[:], pt[:, k * 128:(k + 1) * 128],
                            V[k], start=(k == 0), stop=(k == nk - 1))

                    sl = segs[l][m]
                    has_sin = any(f == SIN for f, _, _ in sl)
                    has_cos = any(f == COS for f, _, _ in sl)
                    has_gauss = any(f == GAUSS for f, _, _ in sl)
                    has_sig = any(f == SIGMOID for f, _, _ in sl)

                    vraw = rpool.tile([128, BS], dt.float32, name="vraw")
                    rt = yt = ut = uct = tmp = tmp2 = None
                    if has_sin or has_cos:
                        rt = spool.tile([128, BS], dt.float32, name="rt")
                        nc.vector._custom_dve(red_op, out=rt[:], in0=ps[:],
                                              s0=-W * INV_2PI, s1=MAGIC,
                                              imm2=TWO_PI_F)
                    if has_sin:
                        ut = spool.tile([128, BS], dt.float32, name="ut")
                        nc.vector._custom_dve(sinu_op, out=ut[:], in0=rt[:],
                                              in1=sincol[:], s0=SINP[0],
                                              s1=SINP[1], imm2=SINP[2])
                    if has_cos:
                        # cos(summed) = cosq(r) directly (even poly)
                        uct = spool.tile([128, BS], dt.float32, name="uct")
                        nc.vector._custom_dve(cosq_op, out=uct[:], in0=rt[:],
                                              in1=coscol[:], s0=COSP[0],
                                              s1=COSP[1], imm2=COSP[2])
                    if has_gauss:
                        # tmp = summed^2 (ACT Square with scale)
                        tmp = spool.tile([128, BS], dt.float32, name="tmp")
                        nc.scalar.activation(tmp[:], ps[:], Act.Square,
                                             scale=-W)
                    if has_sig:
                        # ACT part of sigmoid, full tile (extra rows unread)
                        tmp2 = spool.tile([128, BS], dt.float32, name="tmp2")
                        nc.scalar.activation(tmp2[:], ps[:], Act.Tanh,
                                             scale=-W / 2)

                    # vraw pieces, descending partition order (true owner of
                    # any 32-alignment overlap writes last in program order)
                    for fid, slo, shi in reversed(sl):
                        for lo, hi in _pieces(slo, shi):
                            s = np.s_[lo:hi, :]
                            if fid == COS:
                                nc.gpsimd.tensor_scalar(
                                    vraw[s], uct[s], 1.0, None, Alu.mult)
                            elif fid == SIN:
                                nc.gpsimd.tensor_tensor(
                                    vraw[s], ut[s], rt[s], Alu.mult)
                            elif fid == INVERT:
                                # Pool cannot read PSUM -> DVE
                                nc.vector.tensor_scalar(
                                    vraw[s], ps[s], W, None, Alu.mult)
                            elif fid == LINEAR:
                                nc.vector.tensor_scalar(
                                    vraw[s], ps[s], -W, None, Alu.mult)
                            elif fid == RELU:
                                nc.scalar.activation(
                                    vraw[s], ps[s], Act.Relu, scale=-W)
                            elif fid == SIGMOID:
                                nc.gpsimd.tensor_scalar(
                                    vraw[s], tmp2[s], 0.5, 0.5, Alu.mult,
                                    Alu.add)
                            elif fid == STEP:
                                # +1 iff summed >= 0 iff -ps >= 0; +eps bias
                                # resolves summed==0 to +1 as the ref does
                                nc.scalar.activation(
                                    vraw[s], ps[s], Act.Sign, scale=-1.0,
                                    bias=epscol[lo:hi])
                            elif fid == ABS:
                                nc.scalar.activation(
                                    vraw[s], ps[s], Act.Abs, scale=-W)
                            elif fid == TANH:
                                nc.scalar.activation(
                                    vraw[s], ps[s], Act.Tanh, scale=-W)
                            elif fid == GAUSS:
                                nc.scalar.activation(
                                    vraw[s], tmp[s], Act.Exp, scale=-1.0)
                            else:
                                raise ValueError(fid)

                    if l < L - 1:
                        vt = vpool.tile([128, BS], dt.bfloat16,
                                        name=f"v{4 + 4 * l + m}")
                        nc.vector._custom_dve(quirk_op, out=vt[:],
                                              in0=vraw[:], s0=-1.0, s1=1.0,
                                              imm2=c)
                        new_v[m] = vt[:]
                    else:
                        nc.sync.dma_start(out_d[m * 128:(m + 1) * 128, :],
                                          vraw[:])
                if l < L - 1:
                    # next layer consumes new tiles in production order,
                    # matching _src_rowbase
                    V.extend(new_v[m] for m in range(4))
    nc.compile()
    return nc


_CACHE = {}


def _get_program(segs_key, segs, w):
    key = (segs_key, float(w))
    if key not in _CACHE:
        _CACHE[key] = _build_program(segs, w)
    return _CACHE[key]


def kernel(x, shared_weight, edge_src, edge_dst, act_ids):
    from concourse.bass_utils import run_bass_kernel_spmd

    w = float(np.asarray(shared_weight))
    assert w > 0.0
    a_pack, xpacks, perms, segs = _preprocess(
        np.asarray(x), w, np.asarray(edge_src), np.asarray(edge_dst),
        np.asarray(act_ids))

    segs_key = tuple(tuple(tuple(r) for r in lm) for lseg in segs for lm in lseg)
    nc = _get_program(segs_key, segs, w)

    in_maps = [
        {"xin": xpacks[cid], "amat": a_pack}
        for cid in range(N_CORES)
    ]
    res = run_bass_kernel_spmd(nc, in_maps, core_ids=list(range(N_CORES)))
    out_sorted = np.concatenate([res.results[cid]["out"]
                                 for cid in range(N_CORES)], axis=1)
    out = np.empty_like(out_sorted)
    out[perms[L - 1]] = out_sorted
    return out.astype(np.float32)


# ---------------------------------------------------------------------------
# Host-side numpy emulation of the device program, for fast numerics checks
# (python kernel_selftest) without touching hardware.
def _emulate(x, shared_weight, edge_src, edge_dst, act_ids):
    w = float(np.asarray(shared_weight))
    a_pack, xpacks, perms, segs = _preprocess(
        np.asarray(x), w, np.asarray(edge_src), np.asarray(edge_dst),
        np.asarray(act_ids))
    c = np.float32((1.0 - w) / w)
    W = np.float32(w)
    bf = ml_dtypes.bfloat16
    outs = []
    for cid in range(N_CORES):
        xp = xpacks[cid]
        V = [xp[:, k * BS:(k + 1) * BS] for k in range(4)]
        vraw_last = {}
        for l in range(L):
            nk = NKS[l]
            new_v = {}
            for m in range(4):
                off = PANEL_OFF[l * 4 + m]
                ps = np.zeros((128, BS), np.float32)
                for k in range(nk):
                    A = a_pack[:, off + k * 128: off + (k + 1) * 128]
                    ps += A.astype(np.float32).T @ V[k].astype(np.float32)
                vraw = np.zeros((128, BS), np.float32)
                summed = (-W * ps).astype(np.float32)
                # trig
                q = (ps * np.float32(-w / (2 * np.pi))).astype(np.float32)
                k2 = ((q + np.float32(1.5 * 2**23)).astype(np.float32)
                      - np.float32(1.5 * 2**23)).astype(np.float32)
                r = ((q - k2) * np.float32(2 * np.pi)).astype(np.float32)
                def sinpoly(xx):
                    t = (xx * xx).astype(np.float32)
                    u = np.broadcast_to(np.float32(SINP[3]), xx.shape)
                    for cc in (SINP[2], SINP[1], SINP[0]):
                        u = (u * t + np.float32(cc)).astype(np.float32)
                    return u

                def cospoly(xx):
                    t = (xx * xx).astype(np.float32)
                    u = np.broadcast_to(np.float32(COSP[3]), xx.shape)
                    for cc in (COSP[2], COSP[1], COSP[0]):
                        u = (u * t + np.float32(cc)).astype(np.float32)
                    return u
                for fid, lo, hi in segs[l][m]:
                    s = np.s_[lo:hi]
                    if fid == LINEAR:
                        vraw[s] = summed[s]
                    elif fid == INVERT:
                        vraw[s] = -summed[s]
                    elif fid == RELU:
                        vraw[s] = np.maximum(summed[s], 0)
                    elif fid == STEP:
                        vraw[s] = np.where(-ps[s] + np.float32(1e-30) >= 0,
                                           1.0, -1.0).astype(np.float32)
                    elif fid == ABS:
                        vraw[s] = np.abs(summed[s])
                    elif fid == TANH:
                        vraw[s] = np.tanh(summed[s]).astype(np.float32)
                    elif fid == SIGMOID:
                        t2 = np.tanh(summed[s] / 2).astype(np.float32)
                        vraw[s] = (t2 * np.float32(0.5)
                                   + np.float32(0.5)).astype(np.float32)
                    elif fid == GAUSS:
                        t2 = ((ps[s] * W * W) * ps[s]).astype(np.float32)
                        vraw[s] = np.exp(-t2).astype(np.float32)
                    elif fid == SIN:
                        vraw[s] = (sinpoly(r[s]) * r[s]).astype(np.float32)
                    elif fid == COS:
                        vraw[s] = cospoly(r[s])
                if l < L - 1:
                    va = ((vraw == -1).astype(np.float32) * c
                          - ((vraw == 1).astype(np.float32) * c + vraw))
                    new_v[m] = va.astype(bf)
                else:
                    vraw_last[m] = vraw
            if l < L - 1:
                V.extend(new_v[m] for m in range(4))
        outs.append(np.concatenate([vraw_last[m] for m in range(4)], 0))
    out_sorted = np.concatenate(outs, axis=1)
    out = np.empty_like(out_sorted)
    out[perms[L - 1]] = out_sorted
    return out.astype(np.float32)


# revision 27
# speedup vs baseline: 1.4438x; 1.0537x over previous
"""Trainium2 Bass kernel for nn_CustomModel_12953621365157 (gnn_message_passing).

Strategy
--------
Data-parallel over the batch axis: 8 cores, 512 batch columns each.

Per layer the reference does gather(edge_src) -> 0.5-weight-with-|g|==1-quirk
-> segment_sum(edge_dst) -> per-node activation.  Because the quirk depends
only on the gathered *value*, it folds into the source node:

    v_adj[n] = v[n] + c*(v[n]==1) - c*(v[n]==-1),   c = (1-w)/w
    summed   = w * (A_l @ v_adj)                    A_l[p,n] = #edges n->p

so each layer is a dense [P x N_l] x [N_l x B_s] matmul on the PE.  A_l is
built host-side from the edge lists; counts are small integers so A is
stored bf16 (exact), halving HBM traffic vs fp32.  V tiles are bf16: the
quirk fold runs in fp32 (preserving reference semantics exactly) and only
the final store rounds.  PSUM accumulation stays fp32.

Activations: nodes are pre-sorted by activation id, grouped by the engine
that writes the result tile, so dispatch is a few partition-range
instructions spread over three engines:
  ACT  (single table set exp_and_others, zero swaps): gauss-exp, tanh,
       abs, step (Sign with +eps bias), sigmoid-tanh
  DVE  : gauss-square (stt), trig range reduction (round-to-nearest
       magic-number trick) + odd deg-7 sin polynomial custom ops,
       quirk fold (fp32 compare, bf16 store)
  Pool : sigmoid finish, relu, linear, invert, sin/cos final multiply
Avoiding the Sin ACT table removes all 8 LoadActFuncSet swaps and the
serializing act-order chain of the previous version.

Chunks are processed m=3..0; the next layer's accumulation consumes the
new V tiles in production order so the last-produced tile is needed last,
hiding act->quirk latency behind the first matmuls of the next layer.
"""

import numpy as np
import ml_dtypes

N_IN = 512
P = 512
L = 4
E = 32768
B = 4096
N_CORES = 8
BS = B // N_CORES  # 512 batch columns per core

# activation ids (order matches reference activations_dict)
LINEAR, STEP, SIN, COS, GAUSS, TANH, SIGMOID, ABS, INVERT, RELU = range(10)
# node sort order per layer: expensive multi-op funcs (trig chain, gauss,
# sigmoid) in the low chunks, which are processed first each layer so their
# long act pipelines overlap the layer's remaining matmuls; cheap one-op
# funcs (relu/linear/invert) on top so the per-layer tail is short.
FUNC_ORDER = [SIN, COS, GAUSS, SIGMOID, TANH, ABS, STEP, RELU, LINEAR, INVERT]
ORDER_RANK = {f: i for i, f in enumerate(FUNC_ORDER)}

NKS = [4, 8, 12, 16]                      # src tiles per layer
PANEL_OFF = []                            # column offset of panel (l, m)
_off = 0
for _l in range(L):
    for _m in range(4):
        PANEL_OFF.append(_off)
        _off += NKS[_l] * 128
TOTCOL = _off                             # 20480

# V-tile order: k<4 input chunks; k>=4: chunk (k-4)%4 of layer (k-4)//4
# (chunks are produced m=0..3).


def _src_rowbase(k):
    if k < 4:
        return k * 128
    lp, i = (k - 4) // 4, (k - 4) % 4
    return N_IN + lp * P + i * 128


def _preprocess(x, w, edge_src, edge_dst, act_ids):
    """Host-side: node sort per layer, packed bf16 A panels, quirk fold."""
    c = (1.0 - w) / w

    perms = []      # perms[l][p_sorted] = orig node j
    inv_perms = []  # inv_perms[l][orig j] = p_sorted
    segs = []       # segs[l][m] = list of (func_id, lo, hi) within chunk m
    for l in range(L):
        ids = np.asarray(act_ids[l])
        key = np.array([ORDER_RANK[int(i)] for i in ids])
        perm = np.argsort(key, kind="stable")
        inv = np.empty(P, np.int64)
        inv[perm] = np.arange(P)
        perms.append(perm)
        inv_perms.append(inv)
        ids_sorted = ids[perm]
        layer_segs = []
        for m in range(4):
            chunk = ids_sorted[m * 128:(m + 1) * 128]
            runs = []
            lo = 0
            for i in range(1, 129):
                if i == 128 or chunk[i] != chunk[lo]:
                    runs.append((int(chunk[lo]), lo, i))
                    lo = i
            layer_segs.append(runs)
        segs.append(layer_segs)

    inv_stack = np.stack(inv_perms)  # [L, P]

    # dense per-layer adjacency in sorted coordinates
    a_dense = [np.zeros((N_IN + l * P, P), np.float32) for l in range(L)]
    for l in range(L):
        src = np.asarray(edge_src[l]).astype(np.int64)
        dst = np.asarray(edge_dst[l]).astype(np.int64)
        g = src.copy()
        m = g >= N_IN
        lp = (g[m] - N_IN) // P
        j = (g[m] - N_IN) % P
        g[m] = N_IN + lp * P + inv_stack[lp, j]
        d = inv_perms[l][dst]
        np.add.at(a_dense[l], (g, d), 1.0)

    # pack into [128, TOTCOL]: panel (l, m) col k*128 + j, row p =
    # A_l[src_rowbase(k) + p, m*128 + j]  (k in V-tile order)
    a_pack = np.zeros((128, TOTCOL), np.float32)
    for l in range(L):
        nk = NKS[l]
        for m in range(4):
            off = PANEL_OFF[l * 4 + m]
            for k in range(nk):
                rb = _src_rowbase(k)
                blk = a_dense[l][rb:rb + 128, m * 128:(m + 1) * 128]
                a_pack[:, off + k * 128: off + (k + 1) * 128] = blk
    assert a_pack.max() < 128  # exact in bf16

    xa = x.astype(np.float32)
    if c != 0.0:
        xa = xa + c * (xa == 1.0) - c * (xa == -1.0)
    xin = -xa  # V tiles hold -v_adj
    # pre-swizzled: xpack[cid][p, k*BS + j] = xin[k*128 + p, cid*BS + j]
    xpacks = []
    for cid in range(N_CORES):
        sl = xin[:, cid * BS:(cid + 1) * BS]             # [512, BS]
        xp = sl.reshape(4, 128, BS).transpose(1, 0, 2).reshape(128, 4 * BS)
        xpacks.append(np.ascontiguousarray(xp.astype(ml_dtypes.bfloat16)))

    # per-(layer, chunk) constant columns for the merged activation ops:
    # col (l*4+m)*3 + 0: tanh/sigmoid scale (-w for tanh, -w/2 for sigmoid)
    # col (l*4+m)*3 + 1: relu/linear/invert scale (-w, -w, +w)
    # col (l*4+m)*3 + 2: relu/linear/invert max-bound (0, -big, -big)
    BIGNEG = np.float32(-3.0e38)
    cols = np.zeros((128, L * 4 * 3), np.float32)
    for l in range(L):
        for m in range(4):
            base = (l * 4 + m) * 3
            for fid, lo, hi in segs[l][m]:
                if fid == TANH:
                    cols[lo:hi, base] = -w
                elif fid == SIGMOID:
                    cols[lo:hi, base] = -w / 2
                elif fid == RELU:
                    cols[lo:hi, base + 1] = -w
                    cols[lo:hi, base + 2] = 0.0
                elif fid == LINEAR:
                    cols[lo:hi, base + 1] = -w
                    cols[lo:hi, base + 2] = BIGNEG
                elif fid == INVERT:
                    cols[lo:hi, base + 1] = w
                    cols[lo:hi, base + 2] = BIGNEG
    return (a_pack.astype(ml_dtypes.bfloat16), xpacks, cols, perms, segs)


# odd deg-7 sin minimax coefficients on [-pi-eps, pi+eps]:
# sin(x) ~= x * (S0 + S1 t + S2 t^2 + S3 t^3),  t = x^2
SINP = [9.99876641e-01, -1.66216805e-01, 8.08060368e-03, -1.52742172e-04]
# even deg-6 cos fit on the same range: cos(x) ~= C0 + C1 t + C2 t^2 + C3 t^3
COSP = [9.98937591e-01, -4.96113910e-01, 3.94725721e-02, -9.88522393e-04]

_OPS = None


def _get_custom_ops():
    """Custom DVE ops: quirk fold, trig range reduction, sin polynomial."""
    global _OPS
    if _OPS is not None:
        return _OPS
    import concourse.dve_ops as dve_ops
    from concourse.dve_spec import (Spec, Src0, C0, C1, C2, C3, lower,
                                    _has_src1, _spill_c3_to_src1, Bin)
    from concourse.dve_uop import AluOp, DveOpSpec

    def eq(a, b):
        return Bin(AluOp.IS_EQ, a, b)

    def _register(name, spec):
        if name not in dve_ops._SUB_OPCODE_FOR_NAME:
            row = max(dve_ops._SUB_OPCODE_FOR_NAME.values()) + 1
            assert row < 0x20
            dve_ops._SUB_OPCODE_FOR_NAME[name] = row
        opcode = dve_ops._SUB_OPCODE_FOR_NAME[name]
        shas = {}
        for ver in ("v3", "v4"):
            u = lower(spec, ver=ver)
            shas[ver] = DveOpSpec(name=name, opcode=opcode, uops=u,
                                  rd1_en=_has_src1(spec)).sha(ver)
        op = dve_ops.DveOp(name, spec, subdim=False, uops_sha=shas)
        for i, o in enumerate(dve_ops.OPS):
            if o.name == name:
                dve_ops.OPS[i] = op
                break
        else:
            dve_ops.OPS.append(op)
        dve_ops.CUSTOM_DVE_SPECS[name] = spec
        return op

    # quirk fold: out = -v_adj = c*(v==-1) - (c*(v==1) + v)
    # s0=-1.0, s1=1.0, imm2=c.  bf16 out AP: the fp32 compare runs before
    # the store rounds, preserving reference semantics exactly.
    quirk = _register("QF_G_ANT", Spec(
        body=eq(Src0, C0) * C2 - (eq(Src0, C1) * C2 + Src0),
        reference=lambda in0, in1, s0, s1, imm2: (
            (in0 == np.float32(s0)).astype(np.float32) * np.float32(imm2)
            - ((in0 == np.float32(s1)).astype(np.float32) * np.float32(imm2)
               + in0)),
    ))

    # trig range reduction: q = s0*ps; out = (q - rne(q)) * imm2
    # rne via the magic-number trick (s1 = 1.5*2^23), valid for |q| < 2^22.
    def _red_ref(in0, in1, s0, s1, imm2):
        q = (in0 * np.float32(s0)).astype(np.float32)
        k = ((q + np.float32(s1)).astype(np.float32)
             - np.float32(s1)).astype(np.float32)
        return ((q - k).astype(np.float32) * np.float32(imm2)).astype(np.float32)

    _q = C0 * Src0
    red = _register("TRIGRED_ANT", Spec(
        body=(_q - ((_q + C1) - C1)) * C2,
        reference=_red_ref,
    ))

    # u = ((S3*t + S2)*t + S1)*t + S0, t = x^2; sin(x) ~= u*x.
    # S3 rides C3 -> spilled to in1 (pass a [128,1] tile holding SINP[3]).
    def _sinu_ref(in0, in1, s0, s1, imm2):
        t = (in0 * in0).astype(np.float32)
        u = np.broadcast_to(np.float32(SINP[3]), in0.shape).astype(np.float32)
        for cc in (imm2, s1, s0):
            u = (u * t + np.float32(cc)).astype(np.float32)
        return u

    t = Src0 * Src0
    sinu = _register("SINU_ANT", Spec(
        body=_spill_c3_to_src1(((C3 * t + C2) * t + C1) * t + C0),
        reference=_sinu_ref,
    ))

    # cos(x) ~= ((P3*t + P2)*t + P1)*t + P0, t = x^2 (direct value, no
    # final multiply).  P3 rides C3 -> in1 = [128,1] tile of COSP[3].
    def _cosq_ref(in0, in1, s0, s1, imm2):
        t = (in0 * in0).astype(np.float32)
        u = np.broadcast_to(np.float32(COSP[3]), in0.shape).astype(np.float32)
        for cc in (imm2, s1, s0):
            u = (u * t + np.float32(cc)).astype(np.float32)
        return u

    cosq = _register("COSQ_ANT", Spec(
        body=_spill_c3_to_src1(((C3 * t + C2) * t + C1) * t + C0),
        reference=_cosq_ref,
    ))

    _OPS = (quirk, red, sinu, cosq)
    return _OPS


def _build_program(segs, w):
    import concourse.bacc as bacc
    import concourse.mybir as mybir
    import concourse.tile as tile

    quirk_op, red_op, sinu_op, cosq_op = _get_custom_ops()

    dt = mybir.dt
    Act = mybir.ActivationFunctionType
    Alu = mybir.AluOpType
    W = float(w)
    assert W > 0.0
    c = (1.0 - W) / W

    TWO_PI_F = float(np.float32(2.0 * np.pi))
    INV_2PI = float(np.float32(1.0 / (2.0 * np.pi)))
    PI_F = float(np.float32(np.pi))
    HALF_PI = float(np.float32(np.pi / 2))
    MAGIC = float(np.float32(1.5 * 2 ** 23))

    nc = bacc.Bacc("TRN2", target_bir_lowering=False, debug=False,
                   num_devices=N_CORES)
    xin = nc.dram_tensor("xin", [128, 4 * BS], dt.bfloat16,
                         kind="ExternalInput").ap()
    a_d = nc.dram_tensor("amat", [128, TOTCOL], dt.bfloat16,
                         kind="ExternalInput").ap()
    cols_d = nc.dram_tensor("cols", [128, L * 4 * 3], dt.float32,
                            kind="ExternalInput").ap()
    out_d = nc.dram_tensor("out", [P, BS], dt.float32,
                           kind="ExternalOutput").ap()

    # merge adjacent {SIGMOID,TANH} -> TANHSIG (one ACT Tanh with a
    # per-partition scale column) and {RELU,LINEAR,INVERT} -> RLI (one DVE
    # tensor_scalar mult+max with scale/bound columns)
    TANHSIG, RLI = 100, 101
    _MERGE = {SIGMOID: TANHSIG, TANH: TANHSIG,
              RELU: RLI, LINEAR: RLI, INVERT: RLI}

    def _merge_runs(runs):
        items = []
        for fid, lo, hi in runs:
            mfid = _MERGE.get(fid, fid)
            if items and items[-1][0] == mfid and mfid in (TANHSIG, RLI):
                items[-1] = (mfid, items[-1][1], hi)
            else:
                items.append((mfid, lo, hi))
        return items

    def _pieces(lo, hi):
        p = (lo // 32) * 32
        out = []
        while p < hi:
            end = min(hi, 64) if p == 32 else hi
            out.append((p, end))
            p = end
        return out

    with tile.TileContext(nc) as tc:
        with tc.tile_pool(name="Ap", bufs=1) as apool, \
             tc.tile_pool(name="Vp", bufs=1) as vpool, \
             tc.tile_pool(name="raw", bufs=8) as rpool, \
             tc.tile_pool(name="scr", bufs=3) as spool, \
             tc.tile_pool(name="ps", bufs=8, space="PSUM") as ppool:

            # poly 4th coefficient columns (C3 spill for SINU/COSQ)
            sincol = vpool.tile([128, 1], dt.float32, name="sincol")
            nc.vector.memset(sincol[:], SINP[3])
            coscol = vpool.tile([128, 1], dt.float32, name="coscol")
            nc.vector.memset(coscol[:], COSP[3])
            # +eps bias column for STEP's Sign (resolves summed==0 to +1)
            epscol = vpool.tile([128, 1], dt.float32, name="epscol")
            nc.vector.memset(epscol[:], 1e-30)

            # input node values (already quirk-folded & negated on host)
            xt = vpool.tile([128, 4 * BS], dt.bfloat16, name="xt")
            V = [xt[:, k * BS:(k + 1) * BS] for k in range(4)]

            # merged-op constant columns
            ctile = vpool.tile([128, L * 4 * 3], dt.float32, name="ctile")

            panels = {}
            for l in range(L):
                for m in range(4):
                    panels[(l, m)] = apool.tile([128, NKS[l] * 128],
                                                dt.bfloat16, name=f"a{l}_{m}")

            def _panel_dma(l, m):
                off = PANEL_OFF[l * 4 + m]
                nc.sync.dma_start(panels[(l, m)][:],
                                  a_d[:, off:off + NKS[l] * 128])

            # A panels stream on the SP HWDGE queue in use order; the input
            # tiles + constant columns go through the Pool SWDGE queue whose
            # descriptor generation runs in parallel with HWDGE's.
            _panel_dma(0, 0)
            for k in range(4):
                nc.gpsimd.dma_start(xt[:, k * BS:(k + 1) * BS],
                                    xin[:, k * BS:(k + 1) * BS])
            nc.gpsimd.dma_start(ctile[:], cols_d[:, :])
            for l in range(L):
                for m in range(4):
                    if (l, m) != (0, 0):
                        _panel_dma(l, m)

            # PE p-state warmup: ~3us of dummy matmuls on a memset tile so
            # the tensor engine is at full clock when the real chain starts.
            warm = vpool.tile([128, 512], dt.bfloat16, name="warm")
            nc.vector.memset(warm[:], 0.0)
            for i in range(8):
                wps = ppool.tile([128, BS], dt.float32, name="ps")
                nc.tensor.matmul(wps[:], warm[:, 0:128], warm[:],
                                 start=True, stop=True)

            for l in range(L):
                nk = NKS[l]
                new_v = {}
                for m in range(4):
                    pt = panels[(l, m)]
                    ps = ppool.tile([128, BS], dt.float32, name="ps")
                    for k in range(nk):
                        nc.tensor.matmul(
                            ps[:], pt[:, k * 128:(k + 1) * 128],
                            V[k], start=(k == 0), stop=(k == nk - 1))

                    sl = segs[l][m]
                    has_sin = any(f == SIN for f, _, _ in sl)
                    has_cos = any(f == COS for f, _, _ in sl)
                    has_gauss = any(f == GAUSS for f, _, _ in sl)
                    cbase = (l * 4 + m) * 3

                    vraw = rpool.tile([128, BS], dt.float32, name="vraw")
                    rt = ut = uct = tmp = None
                    if has_sin or has_cos:
                        rt = spool.tile([128, BS], dt.float32, name="rt")
                        nc.vector._custom_dve(red_op, out=rt[:], in0=ps[:],
                                              s0=-W * INV_2PI, s1=MAGIC,
                                              imm2=TWO_PI_F)
                    if has_sin:
                        ut = spool.tile([128, BS], dt.float32, name="ut")
                        nc.vector._custom_dve(sinu_op, out=ut[:], in0=rt[:],
                                              in1=sincol[:], s0=SINP[0],
                                              s1=SINP[1], imm2=SINP[2])
                    if has_cos:
                        # cos(summed) = cosq(r) directly (even poly)
                        uct = spool.tile([128, BS], dt.float32, name="uct")
                        nc.vector._custom_dve(cosq_op, out=uct[:], in0=rt[:],
                                              in1=coscol[:], s0=COSP[0],
                                              s1=COSP[1], imm2=COSP[2])
                    if has_gauss:
                        # tmp = summed^2 (ACT Square with scale)
                        tmp = spool.tile([128, BS], dt.float32, name="tmp")
                        nc.scalar.activation(tmp[:], ps[:], Act.Square,
                                             scale=-W)

                    # vraw pieces, descending partition order (true owner of
                    # any 32-alignment overlap writes last in program order).
                    # The sigmoid finish rides right after the merged
                    # tanh/sigmoid op: its down-extension lands in GAUSS rows
                    # which are rewritten later (descending) by Exp.
                    for mfid, slo, shi in reversed(_merge_runs(sl)):
                        for lo, hi in _pieces(slo, shi):
                            s = np.s_[lo:hi, :]
                            if mfid == COS:
                                nc.gpsimd.tensor_scalar(
                                    vraw[s], uct[s], 1.0, None, Alu.mult)
                            elif mfid == SIN:
                                nc.gpsimd.tensor_tensor(
                                    vraw[s], ut[s], rt[s], Alu.mult)
                            elif mfid == RLI:
                                nc.vector.tensor_scalar(
                                    vraw[s], ps[s],
                                    ctile[lo:hi, cbase + 1:cbase + 2],
                                    ctile[lo:hi, cbase + 2:cbase + 3],
                                    Alu.mult, Alu.max)
                            elif mfid == TANHSIG:
                                nc.scalar.activation(
                                    vraw[s], ps[s], Act.Tanh,
                                    scale=ctile[lo:hi, cbase:cbase + 1])
                            elif mfid == STEP:
                                # +1 iff summed >= 0 iff -ps >= 0; +eps bias
                                # resolves summed==0 to +1 as the ref does
                                nc.scalar.activation(
                                    vraw[s], ps[s], Act.Sign, scale=-1.0,
                                    bias=epscol[lo:hi])
                            elif mfid == ABS:
                                nc.scalar.activation(
                                    vraw[s], ps[s], Act.Abs, scale=-W)
                            elif mfid == GAUSS:
                                nc.scalar.activation(
                                    vraw[s], tmp[s], Act.Exp, scale=-1.0)
                            else:
                                raise ValueError(mfid)
                        if mfid == TANHSIG:
                            for fid2, g_lo, g_hi in sl:
                                if fid2 != SIGMOID:
                                    continue
                                for lo, hi in _pieces(g_lo, g_hi):
                                    s = np.s_[lo:hi, :]
                                    nc.gpsimd.tensor_scalar(
                                        vraw[s], vraw[s], 0.5, 0.5,
                                        Alu.mult, Alu.add)

                    if l < L - 1:
                        vt = vpool.tile([128, BS], dt.bfloat16,
                                        name=f"v{4 + 4 * l + m}")
                        nc.vector._custom_dve(quirk_op, out=vt[:],
                                              in0=vraw[:], s0=-1.0, s1=1.0,
                                              imm2=c)
                        new_v[m] = vt[:]
                    else:
                        nc.sync.dma_start(out_d[m * 128:(m + 1) * 128, :],
                                          vraw[:])
                if l < L - 1:
                    # next layer consumes new tiles in production order,
                    # matching _src_rowbase
                    V.extend(new_v[m] for m in range(4))
    nc.compile()
    return nc


_CACHE = {}


def _get_program(segs_key, segs, w):
    key = (segs_key, float(w))
    if key not in _CACHE:
        _CACHE[key] = _build_program(segs, w)
    return _CACHE[key]


def kernel(x, shared_weight, edge_src, edge_dst, act_ids):
    from concourse.bass_utils import run_bass_kernel_spmd

    w = float(np.asarray(shared_weight))
    assert w > 0.0
    a_pack, xpacks, cols, perms, segs = _preprocess(
        np.asarray(x), w, np.asarray(edge_src), np.asarray(edge_dst),
        np.asarray(act_ids))

    segs_key = tuple(tuple(tuple(r) for r in lm) for lseg in segs for lm in lseg)
    nc = _get_program(segs_key, segs, w)

    in_maps = [
        {"xin": xpacks[cid], "amat": a_pack, "cols": cols}
        for cid in range(N_CORES)
    ]
    res = run_bass_kernel_spmd(nc, in_maps, core_ids=list(range(N_CORES)))
    out_sorted = np.concatenate([res.results[cid]["out"]
                                 for cid in range(N_CORES)], axis=1)
    out = np.empty_like(out_sorted)
    out[perms[L - 1]] = out_sorted
    return out.astype(np.float32)


# ---------------------------------------------------------------------------
# Host-side numpy emulation of the device program, for fast numerics checks
# (python kernel_selftest) without touching hardware.
def _emulate(x, shared_weight, edge_src, edge_dst, act_ids):
    w = float(np.asarray(shared_weight))
    a_pack, xpacks, cols, perms, segs = _preprocess(
        np.asarray(x), w, np.asarray(edge_src), np.asarray(edge_dst),
        np.asarray(act_ids))
    c = np.float32((1.0 - w) / w)
    W = np.float32(w)
    bf = ml_dtypes.bfloat16
    outs = []
    for cid in range(N_CORES):
        xp = xpacks[cid]
        V = [xp[:, k * BS:(k + 1) * BS] for k in range(4)]
        vraw_last = {}
        for l in range(L):
            nk = NKS[l]
            new_v = {}
            for m in range(4):
                off = PANEL_OFF[l * 4 + m]
                ps = np.zeros((128, BS), np.float32)
                for k in range(nk):
                    A = a_pack[:, off + k * 128: off + (k + 1) * 128]
                    ps += A.astype(np.float32).T @ V[k].astype(np.float32)
                vraw = np.zeros((128, BS), np.float32)
                summed = (-W * ps).astype(np.float32)
                # trig
                q = (ps * np.float32(-w / (2 * np.pi))).astype(np.float32)
                k2 = ((q + np.float32(1.5 * 2**23)).astype(np.float32)
                      - np.float32(1.5 * 2**23)).astype(np.float32)
                r = ((q - k2) * np.float32(2 * np.pi)).astype(np.float32)
                def sinpoly(xx):
                    t = (xx * xx).astype(np.float32)
                    u = np.broadcast_to(np.float32(SINP[3]), xx.shape)
                    for cc in (SINP[2], SINP[1], SINP[0]):
                        u = (u * t + np.float32(cc)).astype(np.float32)
                    return u

                def cospoly(xx):
                    t = (xx * xx).astype(np.float32)
                    u = np.broadcast_to(np.float32(COSP[3]), xx.shape)
                    for cc in (COSP[2], COSP[1], COSP[0]):
                        u = (u * t + np.float32(cc)).astype(np.float32)
                    return u
                for fid, lo, hi in segs[l][m]:
                    s = np.s_[lo:hi]
                    if fid == LINEAR:
                        vraw[s] = summed[s]
                    elif fid == INVERT:
                        vraw[s] = -summed[s]
                    elif fid == RELU:
                        vraw[s] = np.maximum(summed[s], 0)
                    elif fid == STEP:
                        vraw[s] = np.where(-ps[s] + np.float32(1e-30) >= 0,
                                           1.0, -1.0).astype(np.float32)
                    elif fid == ABS:
                        vraw[s] = np.abs(summed[s])
                    elif fid == TANH:
                        vraw[s] = np.tanh(summed[s]).astype(np.float32)
                    elif fid == SIGMOID:
                        t2 = np.tanh(summed[s] / 2).astype(np.float32)
                        vraw[s] = (t2 * np.float32(0.5)
                                   + np.float32(0.5)).astype(np.float32)
                    elif fid == GAUSS:
                        t2 = ((ps[s] * W * W) * ps[s]).astype(np.float32)
                        vraw[s] = np.exp(-t2).astype(np.float32)
                    elif fid == SIN:
                        vraw[s] = (sinpoly(r[s]) * r[s]).astype(np.float32)
                    elif fid == COS:
                        vraw[s] = cospoly(r[s])
                if l < L - 1:
                    va = ((vraw == -1).astype(np.float32) * c
                          - ((vraw == 1).astype(np.float32) * c + vraw))
                    new_v[m] = va.astype(bf)
                else:
                    vraw_last[m] = vraw
            if l < L - 1:
                V.extend(new_v[m] for m in range(4))
        outs.append(np.concatenate([vraw_last[m] for m in range(4)], 0))
    out_sorted = np.concatenate(outs, axis=1)
    out = np.empty_like(out_sorted)
    out[perms[L - 1]] = out_sorted
    return out.astype(np.float32)
